# revision 18
# baseline (speedup 1.0000x reference)
"""MHA layer (QKV proj + masked softmax attention + out proj + residual + LayerNorm)
on 8 NeuronCores. Sharding: batch(4) x query-half(2). No collectives: each core
computes K/V for its full batch, Q only for its half of T.

Projections, AV, and out-proj matmuls run in fp8e4m3 DoubleRow perf mode
(2 contraction slabs per pass, half PE cost). Weights are scaled x64 on the
host to keep fp8 values out of the subnormal range; the kernel folds the
inverse scales into existing vector-op scalar slots. Scores stay bf16.

Self-contained: hardcodes shapes from the problem spec.
"""

import numpy as np

import concourse.bass as bass
import concourse.bacc as bacc
import concourse.tile as tile
import concourse.mybir as mybir
from concourse.bass_utils import run_bass_kernel_spmd

B, T, C, H, D = 4, 2048, 1024, 16, 64
TQ = T // 2          # query rows per core
N_CORES = 8
P = 128
NJ = C // P          # 8 c-chunks
NTK = T // P         # 16 key chunks
NPK = NTK // 2       # 8 key-chunk pairs (DoubleRow AV)
LN_EPS = 1e-5
VSLOT = 66           # V_aug per-head slot: 64 V cols + 1 ones + 1 pad
WS = 64.0            # host-side fp8 weight scale

f32 = mybir.dt.float32
f32r = mybir.dt.float32r
bf16 = mybir.dt.bfloat16
fp8 = mybir.dt.float8e4
AX = mybir.AxisListType
ALU = mybir.AluOpType
ACTF = mybir.ActivationFunctionType
DR = mybir.MatmulPerfMode.DoubleRow


def build(affine: bool):
    import os as _os0
    phase_lim = int(_os0.environ.get("K_PHASE", "4"))
    nc = bacc.Bacc("TRN2", target_bir_lowering=False, debug=False,
                   num_devices=N_CORES)

    # x transposed to [C, T] fp8 on the host
    xt8d = nc.dram_tensor("xt8d", [C, T], fp8, kind="ExternalInput")
    # all four weight matrices, x64 (Wq also x 1/sqrt(D)), fp8
    w48 = nc.dram_tensor("w48", [4 * C, C], fp8, kind="ExternalInput")
    # fx rows: 0..TQ-1 xres; TQ+0 bq*sc*64; +1 bk*64; +2 bv; +3 bp;
    #          +4 lng; +5 lnb; +6 mask/64
    fx = nc.dram_tensor("fx", [TQ + 7, C], f32, kind="ExternalInput")
    wq = w48[0 * C:1 * C, :]
    wk = w48[1 * C:2 * C, :]
    wv = w48[2 * C:3 * C, :]
    wp = w48[3 * C:4 * C, :]
    xres = fx[0:TQ, :]
    outd = nc.dram_tensor("out", [TQ, C], f32, kind="ExternalOutput")

    with tile.TileContext(nc) as tc:
        with (
            tc.tile_pool(name="pers", bufs=1) as pers,
            tc.tile_pool(name="big", bufs=1) as bigp,
            tc.tile_pool(name="wbig", bufs=1) as wbigp,
            tc.tile_pool(name="wsl", bufs=4) as wslp,
            tc.tile_pool(name="ev", bufs=2) as evp,
            tc.tile_pool(name="sm", bufs=2) as smp,
            tc.tile_pool(name="psum", bufs=1, space=bass.MemorySpace.PSUM) as psp,
        ):
            # ---- phase A: small loads, broadcasts ----
            mrow_f = smp.tile([1, TQ], f32, tag="sr", name="mrow_f")
            nc.sync.dma_start(mrow_f[:], fx[TQ + 6:TQ + 7, :])
            mrow = pers.tile([1, TQ], bf16, tag="mrow")
            nc.vector.tensor_copy(mrow[:], mrow_f[:])
            bvrow = pers.tile([1, C], f32, tag="bvrow")
            nc.sync.dma_start(bvrow[:], fx[TQ + 2:TQ + 3, :])
            bprow = pers.tile([1, C], f32, tag="bprow")
            nc.sync.dma_start(bprow[:], fx[TQ + 3:TQ + 4, :])
            bq_t = pers.tile([P, NJ], f32, tag="bq_t")
            nc.sync.dma_start(bq_t[:],
                              fx[TQ + 0:TQ + 1, :].rearrange("a (j p) -> p (a j)", p=P))
            bk_t = pers.tile([P, NJ], f32, tag="bk_t")
            nc.sync.dma_start(bk_t[:],
                              fx[TQ + 1:TQ + 2, :].rearrange("a (j p) -> p (a j)", p=P))

            eps_t = pers.tile([P, 1], f32, tag="eps_t")
            nc.gpsimd.memset(eps_t[:], LN_EPS)
            # 1/WS so the den broadcast yields WS/den and yt8 = WS * y
            ones64f = pers.tile([P, 64], f32, tag="ones64f")
            nc.gpsimd.memset(ones64f[:], 1.0 / WS)
            ones64 = pers.tile([P, 64], f32r, tag="ones64")
            nc.vector.tensor_copy(ones64[64:65, :], ones64f[64:65, :])
            mask_bc = pers.tile([P, TQ], bf16, tag="mask_bc")
            nc.gpsimd.partition_broadcast(mask_bc[:], mrow[:])
            bv_bc = pers.tile([P, C], f32, tag="bv_bc")
            nc.gpsimd.partition_broadcast(bv_bc[:], bvrow[:])
            bp_bc = pers.tile([P, C], f32, tag="bp_bc")
            nc.gpsimd.partition_broadcast(bp_bc[:], bprow[:])
            if affine:
                lngrow = pers.tile([1, C], f32, tag="lngrow")
                nc.sync.dma_start(lngrow[:], fx[TQ + 4:TQ + 5, :])
                lnbrow = pers.tile([1, C], f32, tag="lnbrow")
                nc.sync.dma_start(lnbrow[:], fx[TQ + 5:TQ + 6, :])
                lng_bc = pers.tile([P, C], f32, tag="lng_bc")
                nc.gpsimd.partition_broadcast(lng_bc[:], lngrow[:])
                lnb_bc = pers.tile([P, C], f32, tag="lnb_bc")
                nc.gpsimd.partition_broadcast(lnb_bc[:], lnbrow[:])

            # xT8[p, i, t] = x[t, i*128+p], one tile, 8 DMA slices
            xt8 = bigp.tile([P, NJ, T], fp8, tag="xt8")
            for i in range(NJ):
                nc.sync.dma_start(xt8[:, i, :], xt8d[i * P:(i + 1) * P, :])

            # ---- persistent attention operands ----
            qt = [pers.tile([P, TQ], bf16, tag=f"qt{j}", name=f"qt{j}")
                  for j in range(NJ)]
            kt = [pers.tile([P, T], bf16, tag=f"kt{j}", name=f"kt{j}")
                  for j in range(NJ)]
            # V_aug in fp8, paired key chunks (DoubleRow slabs)
            vaug = [pers.tile([P, 2, H * VSLOT], fp8, tag=f"va{m}", name=f"va{m}")
                    for m in range(NPK)]
            # y^T in fp8 (x WS), single tile, slab pairs along j
            yt8 = pers.tile([P, NJ, TQ], fp8, tag="yt8")

            # ---- phase B1: V = x @ Wv + bv -> vaug8 (+ ones cols) ----
            def v_produce():
                wv8 = wbigp.tile([P, 4, 2, C], fp8, tag="wbig8", bufs=2,
                                 name="wv8")
                for mi in range(4):
                    for s in range(2):
                        r = 2 * mi + s
                        nc.sync.dma_start(wv8[:, mi, s, :],
                                          wv[r * P:(r + 1) * P, :])
                for m in range(NPK):
                    va = vaug[m][:].rearrange("p s (h e) -> p s h e", e=VSLOT)
                    nc.gpsimd.memset(va[:, :, :, 64:65], 1.0)
                for tk in range(NTK):
                    for d2 in range(2):
                        m, par = tk // 2, tk % 2
                        psv = psp.tile([P, 512], f32, tag="sc", bufs=2)
                        for mi in range(4):
                            nc.tensor.matmul(
                                psv[:],
                                xt8[:, 2 * mi:2 * mi + 2, tk * P:(tk + 1) * P],
                                wv8[:, mi, :, d2 * 512:(d2 + 1) * 512],
                                start=(mi == 0), stop=(mi == 3),
                                perf_mode=DR)
                        va = vaug[m][:].rearrange("p s (h e) -> p s h e",
                                                  e=VSLOT)
                        nc.vector.scalar_tensor_tensor(
                            va[:, par, 8 * d2:8 * d2 + 8, 0:64],
                            psv[:].rearrange("p (h d) -> p h d", d=D),
                            1.0 / WS,
                            bv_bc[:, d2 * 512:(d2 + 1) * 512].rearrange(
                                "p (h d) -> p h d", d=D),
                            op0=ALU.mult, op1=ALU.add)

            # ---- phase B2 + C: per c-chunk j: Q^T, K^T then attention ----
            def qk_produce(j):
                wq8 = wslp.tile([P, 4, 2, P], fp8, tag="wsl", name=f"wq8_{j}")
                wk8 = wslp.tile([P, 4, 2, P], fp8, tag="wsl", name=f"wk8_{j}")
                for mi in range(4):
                    for s in range(2):
                        r = 2 * mi + s
                        nc.sync.dma_start(
                            wq8[:, mi, s, :],
                            wq[r * P:(r + 1) * P, j * P:(j + 1) * P])
                        nc.sync.dma_start(
                            wk8[:, mi, s, :],
                            wk[r * P:(r + 1) * P, j * P:(j + 1) * P])
                for blk in range(2):
                    psq = psp.tile([P, 512], f32, tag="sc", bufs=2,
                                   name=f"psq{j}_{blk}")
                    for mi in range(4):
                        nc.tensor.matmul(
                            psq[:], wq8[:, mi, :, :],
                            xt8[:, 2 * mi:2 * mi + 2,
                                blk * 512:(blk + 1) * 512],
                            start=(mi == 0), stop=(mi == 3), perf_mode=DR)
                    # qt = (psq + bq64) * (mask/64): masked rows -> 0 scores
                    nc.vector.scalar_tensor_tensor(
                        qt[j][:, blk * 512:(blk + 1) * 512], psq[:],
                        bq_t[:, j:j + 1],
                        mask_bc[:, blk * 512:(blk + 1) * 512],
                        op0=ALU.add, op1=ALU.mult)
                for th in range(2):
                    for blk in range(2):
                        psk = psp.tile([P, 512], f32, tag="sc", bufs=2,
                                       name=f"psk{j}_{th}_{blk}")
                        for mi in range(4):
                            nc.tensor.matmul(
                                psk[:], wk8[:, mi, :, :],
                                xt8[:, 2 * mi:2 * mi + 2,
                                    th * 1024 + blk * 512:
                                    th * 1024 + (blk + 1) * 512],
                                start=(mi == 0), stop=(mi == 3), perf_mode=DR)
                        nc.vector.tensor_scalar(
                            kt[j][:, th * 1024 + blk * 512:
                                     th * 1024 + (blk + 1) * 512], psk[:],
                            bk_t[:, j:j + 1], 1.0 / WS,
                            op0=ALU.add, op1=ALU.mult)

            def attn_chunk(j):
                yaccs = []
                for hh in range(2):
                    ya = psp.tile([65, TQ], f32, tag="yacc", bufs=2,
                                  name=f"yacc{j}_{hh}")
                    yaccs.append(ya)
                for m in range(NPK):
                    ex8 = [None, None]
                    for hh in range(2):
                        ex8[hh] = evp.tile([P, 2, TQ], fp8, tag="ex", bufs=4,
                                           name=f"ex{j}_{hh}")
                    for par in range(2):
                        tk = 2 * m + par
                        for hh in range(2):
                            pb = hh * 64
                            pss = psp.tile([P, 1024], f32, tag="sc", bufs=2,
                                           name=f"pss{j}_{hh}")
                            for blk in range(2):
                                nc.tensor.matmul(
                                    pss[:, blk * 512:(blk + 1) * 512],
                                    kt[j][pb:pb + 64, tk * P:(tk + 1) * P],
                                    qt[j][pb:pb + 64,
                                          blk * 512:(blk + 1) * 512],
                                    start=True, stop=True,
                                    tile_position=(pb, 0))
                            nc.scalar.activation(ex8[hh][:, par, :], pss[:],
                                                 ACTF.Exp)
                    for hh in range(2):
                        h = 2 * j + hh
                        va = vaug[m][:].rearrange("p s (h e) -> p s h e",
                                                  e=VSLOT)
                        for blk in range(2):
                            nc.tensor.matmul(
                                yaccs[hh][:, blk * 512:(blk + 1) * 512],
                                va[:, :, h, 0:65],
                                ex8[hh][:, :, blk * 512:(blk + 1) * 512],
                                start=(m == 0), stop=(m == NPK - 1),
                                perf_mode=DR)
                for hh in range(2):
                    yacc = yaccs[hh]
                    # normalize: row 64 of yacc is the softmax denominator.
                    # den -> SBUF, broadcast via PE ones(=1/WS) outer product,
                    # reciprocal -> WS/den, multiply (yt8 = WS * y).
                    srden = smp.tile([P, TQ], f32r, tag="sr")
                    nc.vector.tensor_copy(srden[64:65, :], yacc[64:65, :])
                    bc = psp.tile([64, TQ], f32, tag="sc", bufs=2,
                                  name=f"bc{j}_{hh}")
                    for blk in range(2):
                        nc.tensor.matmul(
                            bc[:, blk * 512:(blk + 1) * 512],
                            ones64[64:65, :],
                            srden[64:65, blk * 512:(blk + 1) * 512],
                            start=True, stop=True,
                            tile_position=(64, 0))
                    srf = smp.tile([64, TQ], f32, tag="srf", bufs=1)
                    nc.vector.reciprocal(srf[:], bc[:])
                    if hh == 0:
                        nc.vector.tensor_tensor(
                            yt8[0:64, j, :], yacc[0:64, :], srf[:],
                            op=ALU.mult)
                    else:
                        yo = smp.tile([64, TQ], fp8, tag="yo", bufs=1)
                        nc.vector.tensor_tensor(
                            yo[:], yacc[0:64, :], srf[:], op=ALU.mult)
                        nc.sync.dma_start(yt8[64:128, j, :], yo[:])

            # qk(0) first so attn(0) scores/exps overlap the V projection;
            # qk(j+1) ahead of attn(j) keeps the Activation engine fed at
            # each j transition.
            if phase_lim >= 2:
                qk_produce(0)
            if phase_lim >= 1:
                v_produce()
            if phase_lim >= 2:
                for j in range(NJ):
                    if j + 1 < NJ:
                        qk_produce(j + 1)
                    if phase_lim >= 3:
                        attn_chunk(j)

            # ---- phase D: out proj + residual + LayerNorm ----
            if phase_lim >= 4:
                wp8 = wbigp.tile([P, 4, 2, C], fp8, tag="wbig8", bufs=2,
                                 name="wp8")
                for mi in range(4):
                    for s in range(2):
                        r = 2 * mi + s
                        nc.sync.dma_start(wp8[:, mi, s, :],
                                          wp[r * P:(r + 1) * P, :])
                for i in range(T // P // 2):  # 8 row-tiles of our TQ rows
                    xr = bigp.tile([P, C], f32, tag=f"xr{i % 2}", bufs=1,
                                   name=f"xr{i}")
                    nc.sync.dma_start(xr[:], xres[i * P:(i + 1) * P, :])
                    hres = evp.tile([P, C], f32, tag="hres", bufs=2)
                    for half in range(2):
                        pso = psp.tile([P, 512], f32, tag="sc", bufs=2,
                                       name=f"pso{i}_{half}")
                        for mi in range(4):
                            nc.tensor.matmul(
                                pso[:],
                                yt8[:, 2 * mi:2 * mi + 2, i * P:(i + 1) * P],
                                wp8[:, mi, :, half * 512:(half + 1) * 512],
                                start=(mi == 0), stop=(mi == 3), perf_mode=DR)
                        # hres = pso / (WS*WS) + bp
                        nc.vector.scalar_tensor_tensor(
                            hres[:, half * 512:(half + 1) * 512], pso[:],
                            1.0 / (WS * WS),
                            bp_bc[:, half * 512:(half + 1) * 512],
                            op0=ALU.mult, op1=ALU.add)
                    nc.vector.tensor_tensor(hres[:], hres[:], xr[:], op=ALU.add)
                    stat = smp.tile([P, 8], f32, tag="stat")
                    nc.vector.reduce_sum(stat[:, 0:1], hres[:], axis=AX.X)
                    sq = evp.tile([P, C], f32, tag="sq", bufs=2)
                    nc.scalar.activation(sq[:], hres[:], ACTF.Square,
                                         accum_out=stat[:, 1:2])
                    # mu, m2, var
                    nc.vector.tensor_scalar(stat[:, 2:3], stat[:, 0:1],
                                            1.0 / C, None, op0=ALU.mult)
                    nc.vector.tensor_scalar(stat[:, 3:4], stat[:, 1:2],
                                            1.0 / C, None, op0=ALU.mult)
                    nc.vector.tensor_tensor(stat[:, 4:5], stat[:, 2:3],
                                            stat[:, 2:3], op=ALU.mult)
                    nc.vector.tensor_tensor(stat[:, 5:6], stat[:, 3:4],
                                            stat[:, 4:5], op=ALU.subtract)
                    nc.scalar.activation(stat[:, 6:7], stat[:, 5:6], ACTF.Sqrt,
                                         bias=eps_t[:])
                    nc.vector.reciprocal(stat[:, 7:8], stat[:, 6:7])
                    nc.vector.tensor_scalar(hres[:], hres[:], stat[:, 2:3],
                                            stat[:, 7:8], op0=ALU.subtract,
                                            op1=ALU.mult)
                    if affine:
                        nc.vector.tensor_tensor(hres[:], hres[:], lng_bc[:],
                                                op=ALU.mult)
                        nc.vector.tensor_tensor(hres[:], hres[:], lnb_bc[:],
                                                op=ALU.add)
                    nc.sync.dma_start(outd[i * P:(i + 1) * P, :], hres[:])

    nc.compile()
    return nc


_CACHE = {}


def _get_nc(affine: bool):
    if affine not in _CACHE:
        _CACHE[affine] = build(affine)
    return _CACHE[affine]


def _make_in_maps(x, Wq, bq, Wk, bk, Wv, bv, Wp, bp, ln_g, ln_b, mask,
                  affine: bool):
    f8 = mybir.dt.np(fp8)
    sc = np.float32(1.0 / np.sqrt(D))
    w48_h = np.concatenate([
        np.asarray(Wq, np.float32) * (sc * WS), np.asarray(Wk, np.float32) * WS,
        np.asarray(Wv, np.float32) * WS, np.asarray(Wp, np.float32) * WS],
        axis=0).astype(f8)
    x = np.asarray(x, np.float32)
    mask = np.asarray(mask)
    extra = np.stack([
        np.asarray(bq, np.float32) * (sc * WS),
        np.asarray(bk, np.float32) * WS,
        np.asarray(bv, np.float32), np.asarray(bp, np.float32),
        np.asarray(ln_g, np.float32), np.asarray(ln_b, np.float32),
        np.zeros(C, np.float32)], axis=0)
    in_maps = []
    for c in range(N_CORES):
        b, half = c // 2, c % 2
        xb = x[b]
        fx_h = np.empty((TQ + 7, C), np.float32)
        fx_h[0:TQ] = xb[half * TQ:(half + 1) * TQ]
        fx_h[TQ:] = extra
        fx_h[TQ + 6, :] = (mask[b, half * TQ:(half + 1) * TQ] != 0) / WS
        m = {
            "xt8d": np.ascontiguousarray(
                np.roll(xb, -half * TQ, axis=0).T).astype(f8),
            "w48": w48_h,
            "fx": fx_h,
        }
        in_maps.append(m)
    return in_maps


def run(inputs: dict, trace: bool = False):
    ln_g = np.asarray(inputs["ln_g"], np.float32)
    ln_b = np.asarray(inputs["ln_b"], np.float32)
    affine = not (np.all(ln_g == 1.0) and np.all(ln_b == 0.0))
    nc = _get_nc(affine)
    in_maps = _make_in_maps(**inputs, affine=affine)
    res = None
    for attempt in range(3):
        try:
            res = run_bass_kernel_spmd(nc, in_maps, list(range(N_CORES)),
                                       trace=trace)
            break
        except Exception:
            if attempt == 2:
                raise
            import time as _time
            _time.sleep(2.0)
    out = np.empty((B, T, C), np.float32)
    for c in range(N_CORES):
        b, half = c // 2, c % 2
        out[b, half * TQ:(half + 1) * TQ] = res.results[c]["out"]
    return out, res


def kernel(**inputs) -> np.ndarray:
    out, _ = run(inputs, trace=False)
    return out


# revision 33
# speedup vs baseline: 1.9722x; 1.9722x over previous
"""MHA layer (QKV proj + masked softmax attention + out proj + residual + LayerNorm)
on 8 NeuronCores. Sharding: batch(4) x query-half(2). No collectives: each core
computes K/V for its full batch, Q only for its half of T.

Projections, AV, and out-proj matmuls run in fp8e4m3 DoubleRow perf mode
(2 contraction slabs per pass, half PE cost). Weights are scaled x64 on the
host to keep fp8 values out of the subnormal range; the kernel folds the
inverse scales into existing vector-op scalar slots. Scores stay bf16.

Self-contained: hardcodes shapes from the problem spec.
"""

import numpy as np

import concourse.bass as bass
import concourse.bacc as bacc
import concourse.tile as tile
import concourse.mybir as mybir
from concourse.bass_utils import run_bass_kernel_spmd

B, T, C, H, D = 4, 2048, 1024, 16, 64
TQ = T // 2          # query rows per core
N_CORES = 8
P = 128
NJ = C // P          # 8 c-chunks
NTK = T // P         # 16 key chunks
NPK = NTK // 2       # 8 key-chunk pairs (DoubleRow AV)
LN_EPS = 1e-5
VSLOT = 66           # V_aug per-head slot: 64 V cols + 1 ones + 1 pad
WS = 64.0            # host-side fp8 weight scale

f32 = mybir.dt.float32
f32r = mybir.dt.float32r
bf16 = mybir.dt.bfloat16
fp8 = mybir.dt.float8e4
AX = mybir.AxisListType
ALU = mybir.AluOpType
ACTF = mybir.ActivationFunctionType
DR = mybir.MatmulPerfMode.DoubleRow


def build(affine: bool):
    import os as _os0
    phase_lim = int(_os0.environ.get("K_PHASE", "4"))
    nc = bacc.Bacc("TRN2", target_bir_lowering=False, debug=False,
                   num_devices=N_CORES)

    # x transposed to [C, T] fp8 on the host
    xt8d = nc.dram_tensor("xt8d", [C, T], fp8, kind="ExternalInput")
    # all four weight matrices, x64 (Wq also x 1/sqrt(D)), fp8
    w48 = nc.dram_tensor("w48", [4 * C, C], fp8, kind="ExternalInput")
    # fx rows: 0..TQ-1 xres; TQ+0 bq*sc*64; +1 bk*64; +2 bv; +3 bp;
    #          +4 lng; +5 lnb; +6 mask/64
    fx = nc.dram_tensor("fx", [TQ + 7, C], f32, kind="ExternalInput")
    wq = w48[0 * C:1 * C, :]
    wk = w48[1 * C:2 * C, :]
    wv = w48[2 * C:3 * C, :]
    wp = w48[3 * C:4 * C, :]
    xres = fx[0:TQ, :]
    outd = nc.dram_tensor("out", [TQ, C], f32, kind="ExternalOutput")

    with tile.TileContext(nc) as tc:
        with (
            tc.tile_pool(name="pers", bufs=1) as pers,
            tc.tile_pool(name="big", bufs=1) as bigp,
            tc.tile_pool(name="wbig", bufs=1) as wbigp,
            tc.tile_pool(name="wsl", bufs=4) as wslp,
            tc.tile_pool(name="ev", bufs=2) as evp,
            tc.tile_pool(name="sm", bufs=2) as smp,
            tc.tile_pool(name="psum", bufs=1, space=bass.MemorySpace.PSUM) as psp,
        ):
            # ---- phase A: x loads first (they gate qk(0)); small row
            # loads are emitted after the weight DMAs via head_small() ----
            mrow_f = smp.tile([1, TQ], f32, tag="sr", name="mrow_f")
            mrow = pers.tile([1, TQ], bf16, tag="mrow")
            bvrow = pers.tile([1, C], f32, tag="bvrow")
            bprow = pers.tile([1, C], f32, tag="bprow")
            bq_t = pers.tile([P, NJ], f32, tag="bq_t")
            bk_t = pers.tile([P, NJ], f32, tag="bk_t")
            eps_t = pers.tile([P, 1], f32, tag="eps_t")
            ones64f = pers.tile([P, 64], f32, tag="ones64f")
            ones64 = pers.tile([P, 64], f32r, tag="ones64")
            mask_bc = pers.tile([P, TQ], bf16, tag="mask_bc")
            bv_bc = pers.tile([P, C], f32, tag="bv_bc")
            bp_bc = pers.tile([P, C], f32, tag="bp_bc")
            if affine:
                lngrow = pers.tile([1, C], f32, tag="lngrow")
                lnbrow = pers.tile([1, C], f32, tag="lnbrow")
                lng_bc = pers.tile([P, C], f32, tag="lng_bc")
                lnb_bc = pers.tile([P, C], f32, tag="lnb_bc")

            # xT8[p, i, t] = x[t, i*128+p], one tile, 8 DMA slices
            xt8 = bigp.tile([P, NJ, T], fp8, tag="xt8")
            for i in range(NJ):
                nc.sync.dma_start(xt8[:, i, :], xt8d[i * P:(i + 1) * P, :])

            def head_small_pre():
                # inputs qk_produce reads: mask, bq, bk
                nc.sync.dma_start(mrow_f[:], fx[TQ + 6:TQ + 7, :])
                nc.vector.tensor_copy(mrow[:], mrow_f[:])
                nc.gpsimd.partition_broadcast(mask_bc[:], mrow[:])
                nc.sync.dma_start(bq_t[:], fx[TQ + 0:TQ + 1, :].rearrange(
                    "a (j p) -> p (a j)", p=P))
                nc.sync.dma_start(bk_t[:], fx[TQ + 1:TQ + 2, :].rearrange(
                    "a (j p) -> p (a j)", p=P))

            def head_small_post():
                nc.sync.dma_start(bvrow[:], fx[TQ + 2:TQ + 3, :])
                nc.sync.dma_start(bprow[:], fx[TQ + 3:TQ + 4, :])
                nc.gpsimd.memset(eps_t[:], LN_EPS)
                # 1/WS so the den broadcast yields WS/den and yt8 = WS * y
                nc.gpsimd.memset(ones64f[:], 1.0 / WS)
                nc.vector.tensor_copy(ones64[64:65, :], ones64f[64:65, :])
                nc.gpsimd.partition_broadcast(bv_bc[:], bvrow[:])
                nc.gpsimd.partition_broadcast(bp_bc[:], bprow[:])
                if affine:
                    nc.sync.dma_start(lngrow[:], fx[TQ + 4:TQ + 5, :])
                    nc.sync.dma_start(lnbrow[:], fx[TQ + 5:TQ + 6, :])
                    nc.gpsimd.partition_broadcast(lng_bc[:], lngrow[:])
                    nc.gpsimd.partition_broadcast(lnb_bc[:], lnbrow[:])

            # ---- persistent attention operands ----
            qt = [pers.tile([P, TQ], bf16, tag=f"qt{j}", name=f"qt{j}")
                  for j in range(NJ)]
            kt = [pers.tile([P, T], bf16, tag=f"kt{j}", name=f"kt{j}")
                  for j in range(NJ)]
            # V_aug in fp8, paired key chunks (DoubleRow slabs)
            vaug = [pers.tile([P, 2, H * VSLOT], fp8, tag=f"va{m}", name=f"va{m}")
                    for m in range(NPK)]
            # y^T in fp8 (x WS), single tile, slab pairs along j
            yt8 = pers.tile([P, NJ, TQ], fp8, tag="yt8")

            # ---- phase B1: V = x @ Wv + bv -> vaug8 (+ ones cols) ----
            wv8_box = [None]

            def v_head():
                wv8 = wbigp.tile([P, 4, 2, C], fp8, tag="wbig8", bufs=2,
                                 name="wv8")
                wv8_box[0] = wv8
                for mi in range(4):
                    for s in range(2):
                        r = 2 * mi + s
                        nc.sync.dma_start(wv8[:, mi, s, :],
                                          wv[r * P:(r + 1) * P, :])
                for m in range(NPK):
                    va = vaug[m][:].rearrange("p s (h e) -> p s h e", e=VSLOT)
                    nc.gpsimd.memset(va[:, :, :, 64:65], 1.0)

            def v_chunk(m):
                wv8 = wv8_box[0]
                for par in range(2):
                    tk = 2 * m + par
                    for d2 in range(2):
                        psv = psp.tile([P, 512], f32, tag="sc", bufs=2)
                        for mi in range(4):
                            nc.tensor.matmul(
                                psv[:],
                                xt8[:, 2 * mi:2 * mi + 2, tk * P:(tk + 1) * P],
                                wv8[:, mi, :, d2 * 512:(d2 + 1) * 512],
                                start=(mi == 0), stop=(mi == 3),
                                perf_mode=DR)
                        va = vaug[m][:].rearrange("p s (h e) -> p s h e",
                                                  e=VSLOT)
                        nc.vector.scalar_tensor_tensor(
                            va[:, par, 8 * d2:8 * d2 + 8, 0:64],
                            psv[:].rearrange("p (h d) -> p h d", d=D),
                            1.0 / WS,
                            bv_bc[:, d2 * 512:(d2 + 1) * 512].rearrange(
                                "p (h d) -> p h d", d=D),
                            op0=ALU.mult, op1=ALU.add)

            # ---- phase B2 + C: per c-chunk j: Q^T, K^T then attention ----
            def qk_produce(j):
                wq8 = wslp.tile([P, 4, 2, P], fp8, tag="wsl", name=f"wq8_{j}")
                wk8 = wslp.tile([P, 4, 2, P], fp8, tag="wsl", name=f"wk8_{j}")
                for mi in range(4):
                    for s in range(2):
                        r = 2 * mi + s
                        nc.sync.dma_start(
                            wq8[:, mi, s, :],
                            wq[r * P:(r + 1) * P, j * P:(j + 1) * P])
                        nc.sync.dma_start(
                            wk8[:, mi, s, :],
                            wk[r * P:(r + 1) * P, j * P:(j + 1) * P])
                for blk in range(2):
                    psq = psp.tile([P, 512], f32, tag="sc", bufs=2,
                                   name=f"psq{j}_{blk}")
                    for mi in range(4):
                        nc.tensor.matmul(
                            psq[:], wq8[:, mi, :, :],
                            xt8[:, 2 * mi:2 * mi + 2,
                                blk * 512:(blk + 1) * 512],
                            start=(mi == 0), stop=(mi == 3), perf_mode=DR)
                    # qt = (psq + bq64) * (mask/64): masked rows -> 0 scores
                    nc.vector.scalar_tensor_tensor(
                        qt[j][:, blk * 512:(blk + 1) * 512], psq[:],
                        bq_t[:, j:j + 1],
                        mask_bc[:, blk * 512:(blk + 1) * 512],
                        op0=ALU.add, op1=ALU.mult)
                for th in range(2):
                    for blk in range(2):
                        psk = psp.tile([P, 512], f32, tag="sc", bufs=2,
                                       name=f"psk{j}_{th}_{blk}")
                        for mi in range(4):
                            nc.tensor.matmul(
                                psk[:], wk8[:, mi, :, :],
                                xt8[:, 2 * mi:2 * mi + 2,
                                    th * 1024 + blk * 512:
                                    th * 1024 + (blk + 1) * 512],
                                start=(mi == 0), stop=(mi == 3), perf_mode=DR)
                        nc.vector.tensor_scalar(
                            kt[j][:, th * 1024 + blk * 512:
                                     th * 1024 + (blk + 1) * 512], psk[:],
                            bk_t[:, j:j + 1], 1.0 / WS,
                            op0=ALU.add, op1=ALU.mult)

            def attn_chunk(j, vfeed=False, mid=None, norm_prev=None):
                yaccs = []
                for hh in range(2):
                    ya = psp.tile([65, TQ], f32, tag="yacc", bufs=2,
                                  name=f"yacc{j}_{hh}")
                    yaccs.append(ya)
                for m in range(NPK):
                    if m == NPK // 2 and mid is not None:
                        mid()
                    ex8 = [None, None]
                    for hh in range(2):
                        ex8[hh] = evp.tile([P, 2, TQ], fp8, tag="ex", bufs=4,
                                           name=f"ex{j}_{hh}")
                    for par in range(2):
                        tk = 2 * m + par
                        for hh in range(2):
                            pb = hh * 64
                            pss = psp.tile([P, 1024], f32, tag="sc", bufs=2,
                                           name=f"pss{j}_{hh}")
                            for blk in range(2):
                                nc.tensor.matmul(
                                    pss[:, blk * 512:(blk + 1) * 512],
                                    kt[j][pb:pb + 64, tk * P:(tk + 1) * P],
                                    qt[j][pb:pb + 64,
                                          blk * 512:(blk + 1) * 512],
                                    start=True, stop=True,
                                    tile_position=(pb, 0))
                            nc.scalar.activation(ex8[hh][:, par, :], pss[:],
                                                 ACTF.Exp)
                    if m == 0 and norm_prev is not None:
                        norm_prev()
                    if vfeed:
                        v_chunk(m)
                    for hh in range(2):
                        h = 2 * j + hh
                        va = vaug[m][:].rearrange("p s (h e) -> p s h e",
                                                  e=VSLOT)
                        for blk in range(2):
                            nc.tensor.matmul(
                                yaccs[hh][:, blk * 512:(blk + 1) * 512],
                                va[:, :, h, 0:65],
                                ex8[hh][:, :, blk * 512:(blk + 1) * 512],
                                start=(m == 0), stop=(m == NPK - 1),
                                perf_mode=DR)
                return yaccs

            def attn_norm(j, yaccs):
                for hh in range(2):
                    yacc = yaccs[hh]
                    # normalize: row 64 of yacc is the softmax denominator.
                    # den -> SBUF, broadcast via PE ones(=1/WS) outer product,
                    # reciprocal -> WS/den, multiply (yt8 = WS * y).
                    srden = smp.tile([P, TQ], f32r, tag="sr")
                    nc.vector.tensor_copy(srden[64:65, :], yacc[64:65, :])
                    bc = psp.tile([64, TQ], f32, tag="sc", bufs=2,
                                  name=f"bc{j}_{hh}")
                    for blk in range(2):
                        nc.tensor.matmul(
                            bc[:, blk * 512:(blk + 1) * 512],
                            ones64[64:65, :],
                            srden[64:65, blk * 512:(blk + 1) * 512],
                            start=True, stop=True,
                            tile_position=(64, 0))
                    srf = smp.tile([64, TQ], f32, tag="srf", bufs=1)
                    nc.vector.reciprocal(srf[:], bc[:])
                    if hh == 0:
                        nc.vector.tensor_tensor(
                            yt8[0:64, j, :], yacc[0:64, :], srf[:],
                            op=ALU.mult)
                    else:
                        yo = smp.tile([64, TQ], fp8, tag="yo", bufs=1)
                        nc.vector.tensor_tensor(
                            yo[:], yacc[0:64, :], srf[:], op=ALU.mult)
                        nc.sync.dma_start(yt8[64:128, j, :], yo[:])

            # qk(0) first so attn(0) scores/exps overlap the V projection;
            # qk(j+1) ahead of attn(j) keeps the Activation engine fed at
            # each j transition.
            head_small_pre()
            if phase_lim >= 2:
                qk_produce(0)
            head_small_post()
            if phase_lim >= 1:
                v_head()
            if phase_lim >= 2:
                norm_prev = None
                for j in range(NJ):
                    mid = (lambda jn=j + 1: qk_produce(jn)) \
                        if j + 1 < NJ else None
                    if phase_lim >= 3:
                        ya = attn_chunk(j, vfeed=(j == 0), mid=mid,
                                        norm_prev=norm_prev)
                        norm_prev = (lambda jj=j, y=ya: attn_norm(jj, y))
                    else:
                        if mid is not None:
                            mid()
                        if j == 0 and phase_lim >= 1:
                            for m in range(NPK):
                                v_chunk(m)
                if norm_prev is not None:
                    norm_prev()
            elif phase_lim >= 1:
                for m in range(NPK):
                    v_chunk(m)

            # ---- phase D: out proj + residual + LayerNorm ----
            if phase_lim >= 4:
                wp8 = wbigp.tile([P, 4, 2, C], fp8, tag="wbig8", bufs=2,
                                 name="wp8")
                for mi in range(4):
                    for s in range(2):
                        r = 2 * mi + s
                        nc.sync.dma_start(wp8[:, mi, s, :],
                                          wp[r * P:(r + 1) * P, :])
                for i in range(T // P // 2):  # 8 row-tiles of our TQ rows
                    xr = bigp.tile([P, C], f32, tag=f"xr{i % 2}", bufs=1,
                                   name=f"xr{i}")
                    nc.sync.dma_start(xr[:], xres[i * P:(i + 1) * P, :])
                    hres = evp.tile([P, C], f32, tag="hres", bufs=2)
                    for half in range(2):
                        pso = psp.tile([P, 512], f32, tag="sc", bufs=2,
                                       name=f"pso{i}_{half}")
                        for mi in range(4):
                            nc.tensor.matmul(
                                pso[:],
                                yt8[:, 2 * mi:2 * mi + 2, i * P:(i + 1) * P],
                                wp8[:, mi, :, half * 512:(half + 1) * 512],
                                start=(mi == 0), stop=(mi == 3), perf_mode=DR)
                        # hres = pso / (WS*WS) + bp
                        nc.vector.scalar_tensor_tensor(
                            hres[:, half * 512:(half + 1) * 512], pso[:],
                            1.0 / (WS * WS),
                            bp_bc[:, half * 512:(half + 1) * 512],
                            op0=ALU.mult, op1=ALU.add)
                    nc.vector.tensor_tensor(hres[:], hres[:], xr[:], op=ALU.add)
                    stat = smp.tile([P, 8], f32, tag="stat")
                    nc.vector.reduce_sum(stat[:, 0:1], hres[:], axis=AX.X)
                    sq = evp.tile([P, C], f32, tag="sq", bufs=2)
                    nc.scalar.activation(sq[:], hres[:], ACTF.Square,
                                         accum_out=stat[:, 1:2])
                    # mu, m2, var
                    nc.vector.tensor_scalar(stat[:, 2:3], stat[:, 0:1],
                                            1.0 / C, None, op0=ALU.mult)
                    nc.vector.tensor_scalar(stat[:, 3:4], stat[:, 1:2],
                                            1.0 / C, None, op0=ALU.mult)
                    nc.vector.tensor_tensor(stat[:, 4:5], stat[:, 2:3],
                                            stat[:, 2:3], op=ALU.mult)
                    nc.vector.tensor_tensor(stat[:, 5:6], stat[:, 3:4],
                                            stat[:, 4:5], op=ALU.subtract)
                    nc.scalar.activation(stat[:, 6:7], stat[:, 5:6], ACTF.Sqrt,
                                         bias=eps_t[:])
                    nc.vector.reciprocal(stat[:, 7:8], stat[:, 6:7])
                    nc.vector.tensor_scalar(hres[:], hres[:], stat[:, 2:3],
                                            stat[:, 7:8], op0=ALU.subtract,
                                            op1=ALU.mult)
                    if affine:
                        nc.vector.tensor_tensor(hres[:], hres[:], lng_bc[:],
                                                op=ALU.mult)
                        nc.vector.tensor_tensor(hres[:], hres[:], lnb_bc[:],
                                                op=ALU.add)
                    nc.sync.dma_start(outd[i * P:(i + 1) * P, :], hres[:])

    nc.compile()
    return nc


_CACHE = {}


def _get_nc(affine: bool):
    if affine not in _CACHE:
        _CACHE[affine] = build(affine)
    return _CACHE[affine]


def _make_in_maps(x, Wq, bq, Wk, bk, Wv, bv, Wp, bp, ln_g, ln_b, mask,
                  affine: bool):
    f8 = mybir.dt.np(fp8)
    sc = np.float32(1.0 / np.sqrt(D))
    w48_h = np.concatenate([
        np.asarray(Wq, np.float32) * (sc * WS), np.asarray(Wk, np.float32) * WS,
        np.asarray(Wv, np.float32) * WS, np.asarray(Wp, np.float32) * WS],
        axis=0).astype(f8)
    x = np.asarray(x, np.float32)
    mask = np.asarray(mask)
    extra = np.stack([
        np.asarray(bq, np.float32) * (sc * WS),
        np.asarray(bk, np.float32) * WS,
        np.asarray(bv, np.float32), np.asarray(bp, np.float32),
        np.asarray(ln_g, np.float32), np.asarray(ln_b, np.float32),
        np.zeros(C, np.float32)], axis=0)
    in_maps = []
    for c in range(N_CORES):
        b, half = c // 2, c % 2
        xb = x[b]
        fx_h = np.empty((TQ + 7, C), np.float32)
        fx_h[0:TQ] = xb[half * TQ:(half + 1) * TQ]
        fx_h[TQ:] = extra
        fx_h[TQ + 6, :] = (mask[b, half * TQ:(half + 1) * TQ] != 0) / WS
        m = {
            "xt8d": np.ascontiguousarray(
                np.roll(xb, -half * TQ, axis=0).T).astype(f8),
            "w48": w48_h,
            "fx": fx_h,
        }
        in_maps.append(m)
    return in_maps


def run(inputs: dict, trace: bool = False):
    ln_g = np.asarray(inputs["ln_g"], np.float32)
    ln_b = np.asarray(inputs["ln_b"], np.float32)
    affine = not (np.all(ln_g == 1.0) and np.all(ln_b == 0.0))
    nc = _get_nc(affine)
    in_maps = _make_in_maps(**inputs, affine=affine)
    res = None
    for attempt in range(3):
        try:
            res = run_bass_kernel_spmd(nc, in_maps, list(range(N_CORES)),
                                       trace=trace)
            break
        except Exception:
            if attempt == 2:
                raise
            import time as _time
            _time.sleep(2.0)
    out = np.empty((B, T, C), np.float32)
    for c in range(N_CORES):
        b, half = c // 2, c % 2
        out[b, half * TQ:(half + 1) * TQ] = res.results[c]["out"]
    return out, res


def kernel(**inputs) -> np.ndarray:
    out, _ = run(inputs, trace=False)
    return out
